# revision 72
# baseline (speedup 1.0000x reference)
"""Trainium2 Bass kernel for MetaPathClassifier (heterogeneous-path GRU).

Strategy (data-parallel over 8 cores, 512 paths each):

Host prep: sort paths by length (desc) and deal round-robin so every core
sees a near-identical length profile; compute per-step column capacities
caps[l] (multiples of 64, shared across cores).  Gather + type-split +
transpose the node features on the host into one feature-major table
  gt2 [128, 2*S']: cols 0:S' paper features (zero for non-paper slots);
  cols S':2S' rows 0:101 = author 0:64 | venue 64:96 | one-hot type
  96:99 | invalid flag 99 | ones 100.
Step-major layout: step l owns columns [base_l, base_l+caps[l]).
Combined weights (W_ih @ W_t).T, per-type bias rows (W_ih@b_t + b_ih), a
BIG-z row (invalid slots saturate the z gate so h freezes exactly), and a
b_hh row (via the ones-row) are extra contraction rows, so the r/z gate
pre-activations come out of PSUM complete - no bias adds on the gate path.

Device: per column stream of each step (<=256 wide early, <=128 late so
independent streams pipeline across steps), x-side and h-side GEMMs
accumulate into one PSUM bank per gate pair (m-tile halves side by side),
ACT applies sigmoid/tanh straight from PSUM, DVE forms the n-gate product
and the convex h update h += z'*(n-h) in bf16 (2x mode).  One PSUM
accumulation group per bank (zero regions are bank-granular), opened by
the first x matmul (all x operands arrive via one DMA semaphore) or the
rank-1 b_hh fold (phn).  Software pipelined: chunk i+1's x-side GEMMs are
emitted before chunk i's h-side so PE never idles.  All matmuls bf16.
Logits are classified incrementally as columns retire.
"""

import contextlib

import numpy as np

import concourse.bacc as bacc
import concourse.mybir as mybir
import concourse.tile as tile
from concourse.bass_utils import run_bass_kernel_spmd

F32 = mybir.dt.float32
F32R = mybir.dt.float32r
BF16 = mybir.dt.bfloat16
AF = mybir.ActivationFunctionType
OP = mybir.AluOpType
NPBF16 = mybir.dt.np(BF16)

NCORES = 8
B, L, H, C = 4096, 8, 256, 8
NB = B // NCORES            # paths per core
G = 3 * H                   # gate width 768
DP, DA, DV = 128, 64, 32
KAV = 101                   # author 64 + venue 32 + onehot 3 + inv 1 + ones 1
R_OH, R_INV, R_ONE = 96, 99, 100
BIGZ = 30000.0              # sigmoid(-(x+BIGZ)) == 0.0 exactly in fp32
CH = 256                    # column chunk width


def default_caps():
    return [512, 448, 384, 320, 256, 192, 128, 64]


def chunks_of(caps):
    """(l, start-in-caps-layout, width) per stream + its gt2 block start.
    gt2 interleaves per-chunk [paper w | av w] blocks so one contiguous
    DMA carries a chunk's whole x-side (single completion semaphore)."""
    bases = np.concatenate([[0], np.cumsum(caps)]).astype(int)
    out = []
    for l, cap in enumerate(caps):
        c0 = 0
        for w in stream_widths(l, cap):
            g0 = int(bases[l]) + c0
            out.append((l, g0, w))
            c0 += w
    return out


def stream_widths(l, cap):
    """Column-stream widths for one step: <=256 wide early (throughput),
    <=128 for the late latency-bound steps (independent streams pipeline
    across steps since the recurrence is per-column)."""
    wmax = CH if l < 4 else 128
    out = []
    rem = cap
    while rem > 0:
        w = min(wmax, rem)
        out.append(w)
        rem -= w
    return out


def build_nc(caps=None, loop=0):
    """loop>0 wraps the whole body in a device-side For_i executing it
    `loop` times back-to-back - used only for wall-clock timing."""
    caps = list(caps) if caps is not None else default_caps()
    assert caps[0] == NB and all(c % 64 == 0 for c in caps)
    bases = np.concatenate([[0], np.cumsum(caps)]).astype(int)
    SP = int(bases[-1])
    chunks = chunks_of(caps)

    nc = bacc.Bacc("TRN2", target_bir_lowering=False, debug=False,
                   num_devices=NCORES)

    def din(name, shape, dt):
        return nc.dram_tensor(name, shape, dt, kind="ExternalInput").ap()

    gtp_d = din("gtp", [DP, SP], BF16)
    gtav_d = din("gtav", [KAV, SP], BF16)
    wx_d = din("wx", [DP, 2 * G], BF16)      # wc1 | lhs2 (zero-padded rows)
    whh_d = din("whh", [128, 2 * G], BF16)   # W_hh.T halves side by side
    swc_d = din("swc", [128, 2 * C], BF16)   # Wc.T halves side by side
    bhb_d = din("bhb", [128, 7], F32)        # bhh6 | bc (rows 0:C of col 6)
    bhr_d = din("bhr", [33, 128], BF16)      # b_hh n-gate rows at 0 and 32
    out_d = nc.dram_tensor("logitsT", [C, NB], F32, kind="ExternalOutput").ap()

    with tile.TileContext(nc) as tc:
        pers = tc.alloc_tile_pool(name="pers", bufs=1)

        def T(shape, dt, name):
            return pers.tile(shape, dt, tag=name, name=name)

        gTp = T([DP, SP], BF16, "gTp")
        gTav = T([KAV, SP], BF16, "gTav")
        wx = T([DP, 2 * G], BF16, "wx")       # wc1 | lhs2
        swhh2 = T([128, 2 * G], BF16, "swhh2")
        swc2 = T([128, 2 * C], BF16, "swc2")
        bhb = T([128, 7], F32, "bhb")
        bhr = T([33, 128], BF16, "bhr")
        ones = T([33, NB], BF16, "ones")
        zlh = T([1, 512], BF16, "zlh")
        hT = T([128, 2 * NB], BF16, "hT")     # cols 0:512 mt0 | 512:1024 mt1
        lsb = T([C, NB], F32, "lsb")

        with (
            tc.tile_pool(name="pr", bufs=2, space="PSUM") as ppr,
            tc.tile_pool(name="pz", bufs=2, space="PSUM") as ppz,
            tc.tile_pool(name="phn", bufs=2, space="PSUM") as pphn,
            tc.tile_pool(name="pxn", bufs=2, space="PSUM") as ppxn,
            tc.tile_pool(name="gate", bufs=4) as gp,
        ):
            # ------ engine warmup: PE p-state ramp + ACT table load + the
            # ------ constants the recurrence needs (before any timing loop)
            with tc.tile_pool(name="warm", bufs=1) as wp:
                wsrc = wp.tile([128, CH], F32R, tag="wsrc")
                wps = ppr.tile([128, 512], F32, tag="pr", name="warmp")
                wact = wp.tile([128, 16], F32, tag="wact")
                nc.vector.memset(wsrc[:].bitcast(F32), 0.0)
                nc.vector.memset(zlh[:], 0.0)
                nc.vector.memset(wact[:], 0.0)
                nc.vector.memset(ones[:], 1.0)
                nc.scalar.activation(wact[:], wact[:], AF.Sigmoid)
                nc.scalar.activation(wact[:], wact[:], AF.Tanh)
                nc.gpsimd.tensor_add(wact[:], wact[:], wact[:])
                for w in range(14):
                    nc.tensor.matmul(wps[:, 0:CH], wsrc[:, 0:128], wsrc[:],
                                     start=(w == 0), stop=(w == 13))

            if True:
                # -------- input DMAs: gt2 + wx on the SP/HWDGE queue in
                # priority order; later weights descriptor-gen on Pool
                def gt_dma(c0, c1):
                    sl = slice(int(c0), int(c1))
                    nc.sync.dma_start(gTp[:, sl], gtp_d[:, sl])
                    nc.sync.dma_start(gTav[:, sl], gtav_d[:, sl])

                nc.gpsimd.dma_start(swc2[:], swc_d[:, :])
                nc.gpsimd.dma_start(bhb[:], bhb_d[:, :])
                nc.gpsimd.dma_start(bhr[:], bhr_d[:, :])
                nc.sync.dma_start(wx[:], wx_d[:, :])
                gt_dma(0, CH)
                gt_dma(CH, bases[1])
                nc.sync.dma_start(swhh2[:], whh_d[:, :])
                gt_dma(bases[1], bases[min(3, len(caps))])
                if len(caps) > 3:
                    gt_dma(bases[3], SP)

                # -------- recurrence, software pipelined.  One PSUM
                # accumulation group per bank (zero regions are 2KB/bank),
                # opened by each tile's first matmul, whose operands arrive
                # via a single DMA semaphore so emission order holds.
                def p_open(p, w):
                    nc.tensor.matmul(p[0:128, 0:w], zlh[0:1, 0:128],
                                     zlh[0:1, 0:w], start=True, stop=False)

                def emit_x(i):
                    l, g0, w = chunks[i]
                    cs = slice(g0, g0 + w)
                    av = slice(g0, g0 + w)
                    pr = ppr.tile([128, 512], F32, tag="pr", name=f"pr{i}")
                    pz = ppz.tile([128, 512], F32, tag="pz", name=f"pz{i}")
                    pxn = ppxn.tile([128, 512], F32, tag="pxn", name=f"px{i}")
                    for p in (pr, pz, pxn):
                        p_open(p, w)
                    for half in range(2):
                        po = slice(half * w, (half + 1) * w)
                        for gi, p in ((0, pr), (1, pz), (2, pxn)):
                            m0 = (2 * gi + half) * 128
                            last = half == 1 and (l == 0 or gi == 2)
                            nc.tensor.matmul(p[:, po], wx[:, m0:m0 + 128],
                                             gTp[:, cs], start=False,
                                             stop=False)
                            nc.tensor.matmul(p[:, po],
                                             wx[0:KAV, G + m0:G + m0 + 128],
                                             gTav[:, av], start=False,
                                             stop=last)
                    return pr, pz, pxn

                def emit_h_gates(i, pr, pz, pxn):
                    l, g0, w = chunks[i]
                    c0 = g0 - int(bases[l])          # path-column offset
                    hs = [slice(k * NB + c0, k * NB + c0 + w)
                          for k in range(2)]
                    phn = None
                    if l > 0:
                        phn = pphn.tile([128, 512], F32, tag="phn",
                                        name=f"ph{i}")
                        p_open(phn, w)
                        # r first (heads the gate chain), then n (feeds
                        # tt), z last (consumed latest)
                        for gi, p in ((0, pr), (2, phn), (1, pz)):
                            for half in range(2):
                                po = slice(half * w, (half + 1) * w)
                                m0 = (2 * gi + half) * 128
                                for k in range(2):
                                    nc.tensor.matmul(
                                        p[:, po],
                                        swhh2[:,
                                              k * G + m0:k * G + m0 + 128],
                                        hT[:, hs[k]],
                                        start=False,
                                        stop=(k == 1 and half == 1
                                              and gi != 2))
                                if gi == 2:
                                    nc.tensor.matmul(
                                        phn[:, po],
                                        bhr[32 * half:32 * half + 1, :],
                                        ones[32 * half:32 * half + 1, 0:w],
                                        start=False, stop=(half == 1))
                    rr = gp.tile([128, 512], BF16, tag="rr", name=f"rr{i}")
                    zz = gp.tile([128, 512], BF16, tag="zz", name=f"zz{i}")
                    tt = gp.tile([128, 512], BF16, tag="tt", name=f"tt{i}")
                    nn = gp.tile([128, 512], BF16, tag="nn", name=f"nn{i}")
                    po = [slice(h_ * w, (h_ + 1) * w) for h_ in range(2)]
                    p2 = slice(0, 2 * w)
                    nc.scalar.activation(rr[:, p2], pr[:, p2], AF.Sigmoid)
                    nc.scalar.activation(zz[:, p2], pz[:, p2], AF.Sigmoid,
                                         scale=-1.0)
                    if l > 0:
                        nc.vector.tensor_mul(tt[:, p2], phn[:, p2],
                                             rr[:, p2])
                    else:
                        for h_ in range(2):
                            nc.vector.tensor_scalar_mul(
                                tt[:, po[h_]], rr[:, po[h_]],
                                bhb[:, 4 + h_:5 + h_])
                    nc.vector.tensor_add(pxn[:, p2], pxn[:, p2], tt[:, p2])
                    nc.scalar.activation(nn[:, p2], pxn[:, p2], AF.Tanh)
                    # h' = h + z'*(n - h)   (z' = 1-z); at l=0: h = z'*n
                    tm = gp.tile([128, 512], BF16, tag="tm", name=f"tm{i}")
                    for h_, eng in ((0, nc.vector), (1, nc.vector)):
                        p_ = po[h_]
                        hv = hT[:, h_ * NB + c0:h_ * NB + c0 + w]
                        if l == 0:
                            eng.tensor_mul(hv, zz[:, p_], nn[:, p_])
                        else:
                            eng.tensor_sub(tm[:, p_], nn[:, p_], hv)
                            eng.tensor_mul(tm[:, p_], zz[:, p_], tm[:, p_])
                            eng.tensor_add(hv, hv, tm[:, p_])

                # classify columns [lo, hi) whose h became final at step l
                def emit_cls(lo, hi):
                    pl = ppr.tile([128, 512], F32, tag="pr", name=f"pl{lo}")
                    for k in range(2):
                        nc.tensor.matmul(pl[0:C, lo:hi],
                                         swc2[:, k * C:(k + 1) * C],
                                         hT[:, k * NB + lo:k * NB + hi],
                                         start=(k == 0), stop=(k == 1))
                    nc.vector.tensor_scalar(lsb[:, lo:hi], pl[0:C, lo:hi],
                                            bhb[0:C, 6:7], None, op0=OP.add)
                    nc.sync.dma_start(out_d[:, lo:hi], lsb[:, lo:hi])

                caps1 = caps + [0]
                pend = emit_x(0)
                for i in range(len(chunks)):
                    nxt = emit_x(i + 1) if i + 1 < len(chunks) else None
                    emit_h_gates(i, *pend)
                    pend = nxt
                    l = chunks[i][0]
                    last = i + 1 == len(chunks) or chunks[i + 1][0] != l
                    if not last:
                        continue
                    if len(caps) < 5:    # degenerate: one classify at end
                        if i + 1 == len(chunks):
                            emit_cls(0, NB)
                    elif l == 3 and caps1[4] < NB:
                        emit_cls(caps1[4], NB)
                    elif l > 3 and caps1[l + 1] < caps1[l]:
                        emit_cls(caps1[l + 1], caps1[l])
                    elif i + 1 == len(chunks):
                        emit_cls(0, caps1[l])

        pers.release()

    nc.finalize()
    return nc


# ---------------------------------------------------------------- host side

def compute_caps(lengths, perm):
    maxc = np.zeros(L, np.int64)
    for c in range(NCORES):
        plen = lengths[perm[c::NCORES]]
        for l in range(L):
            maxc[l] = max(maxc[l], int((plen > l).sum()))
    caps = [NB]
    for l in range(1, L):
        caps.append(int(min(NB, 64 * -(-maxc[l] // 64))))
    while caps and caps[-1] == 0:
        caps.pop()
    return caps


def make_in_maps(inputs):
    f32 = lambda k: np.asarray(inputs[k], dtype=np.float32)
    i64 = lambda k: np.asarray(inputs[k]).astype(np.int64)
    W_ih, W_hh = f32("W_ih"), f32("W_hh")
    b_ih, b_hh = f32("b_ih"), f32("b_hh")
    paper, author, venue = f32("paper_x"), f32("author_x"), f32("venue_x")
    lengths, type_ids, node_ids = (i64("lengths"), i64("type_ids"),
                                   i64("node_ids"))

    perm = np.argsort(-lengths, kind="stable")
    caps = compute_caps(lengths, perm)
    bases = np.concatenate([[0], np.cumsum(caps)]).astype(int)
    SP = int(bases[-1])

    wx = np.zeros((DP, 2 * G), np.float32)
    wx[:, 0:G] = (W_ih @ f32("Wp")).T
    wx[0:DA, G:2 * G] = (W_ih @ f32("Wa")).T
    wx[DA:DA + DV, G:2 * G] = (W_ih @ f32("Wv")).T
    for t, bk in enumerate(("bp", "ba", "bv")):
        wx[R_OH + t, G:2 * G] = W_ih @ f32(bk) + b_ih
    wx[R_INV, G + H:G + 2 * H] = BIGZ
    wx[R_ONE, G:G + 2 * H] = b_hh[0:2 * H]
    whh = np.zeros((128, 2 * G), np.float32)
    whh[:, 0:G] = W_hh.T[0:128]
    whh[:, G:2 * G] = W_hh.T[128:256]
    swc = np.zeros((128, 2 * C), np.float32)
    swc[:, 0:C] = f32("Wc").T[0:128]
    swc[:, C:2 * C] = f32("Wc").T[128:256]
    bhr = np.zeros((33, 128), np.float32)
    bhr[0] = b_hh[2 * H:2 * H + 128]
    bhr[32] = b_hh[2 * H + 128:3 * H]
    bhb = np.zeros((128, 7), np.float32)
    bhb[:, 0:6] = b_hh.reshape(6, 128).T
    bhb[0:C, 6] = f32("bc")
    shared = {
        "wx": np.ascontiguousarray(wx.astype(NPBF16)),
        "whh": np.ascontiguousarray(whh.astype(NPBF16)),
        "swc": np.ascontiguousarray(swc.astype(NPBF16)),
        "bhb": np.ascontiguousarray(bhb),
        "bhr": np.ascontiguousarray(bhr.astype(NPBF16)),
    }

    in_maps = []
    for c in range(NCORES):
        pids = perm[c::NCORES]               # original path ids, desc length
        plen = lengths[pids]
        gtp = np.zeros((SP, DP), np.float32)     # slot-major; transposed below
        gtav = np.zeros((SP, KAV), np.float32)
        for l in range(len(caps)):
            b0, cap = int(bases[l]), caps[l]
            idx = pids[:cap]
            valid = plen[:cap] > l
            t = type_ids[idx, l]
            nid = node_ids[idx, l]
            rows = b0 + np.arange(cap)
            pm = valid & (t == 0)
            am = valid & (t == 1)
            vm = valid & (t == 2)
            gtp[rows[pm]] = paper[nid[pm]]
            gtav[rows[am], 0:DA] = author[nid[am]]
            gtav[rows[vm], DA:DA + DV] = venue[nid[vm]]
            gtav[rows, R_OH] = pm
            gtav[rows, R_OH + 1] = am
            gtav[rows, R_OH + 2] = vm
            gtav[rows, R_INV] = ~valid
            gtav[rows, R_ONE] = 1.0
        m = dict(shared)
        m["gtp"] = np.ascontiguousarray(gtp.T.astype(NPBF16))
        m["gtav"] = np.ascontiguousarray(gtav.T.astype(NPBF16))
        in_maps.append(m)
    return in_maps, perm, caps


_NC_CACHE = {}


def _get_nc(caps=None, loop=0):
    key = (tuple(caps) if caps is not None else tuple(default_caps()), loop)
    if key not in _NC_CACHE:
        _NC_CACHE[key] = build_nc(list(key[0]), loop=loop)
    return _NC_CACHE[key]


def kernel(**inputs) -> np.ndarray:
    in_maps, perm, caps = make_in_maps(inputs)
    nc = _get_nc(caps)
    res = run_bass_kernel_spmd(nc, in_maps, core_ids=list(range(NCORES)))
    out = np.empty((B, C), np.float32)
    for c in range(NCORES):
        lt = np.asarray(res.results[c]["logitsT"])    # [C, NB]
        out[perm[c::NCORES]] = lt.T
    return np.ascontiguousarray(out)


# revision 74
# speedup vs baseline: 1.0112x; 1.0112x over previous
"""Trainium2 Bass kernel for MetaPathClassifier (heterogeneous-path GRU).

Strategy (data-parallel over 8 cores, 512 paths each):

Host prep: sort paths by length (desc) and deal round-robin so every core
sees a near-identical length profile; compute per-step column capacities
caps[l] (multiples of 64, shared across cores).  Gather + type-split +
transpose the node features on the host into one feature-major table
  gt2 [128, 2*S']: cols 0:S' paper features (zero for non-paper slots);
  cols S':2S' rows 0:101 = author 0:64 | venue 64:96 | one-hot type
  96:99 | invalid flag 99 | ones 100.
Step-major layout: step l owns columns [base_l, base_l+caps[l]).
Combined weights (W_ih @ W_t).T, per-type bias rows (W_ih@b_t + b_ih), a
BIG-z row (invalid slots saturate the z gate so h freezes exactly), and a
b_hh row (via the ones-row) are extra contraction rows, so the r/z gate
pre-activations come out of PSUM complete - no bias adds on the gate path.

Device: per column stream of each step (<=256 wide early, <=128 late so
independent streams pipeline across steps), x-side and h-side GEMMs
accumulate into one PSUM bank per gate pair (m-tile halves side by side),
ACT applies sigmoid/tanh straight from PSUM, DVE forms the n-gate product
and the convex h update h += z'*(n-h) in bf16 (2x mode).  One PSUM
accumulation group per bank (zero regions are bank-granular), opened by
the first x matmul (all x operands arrive via one DMA semaphore) or the
rank-1 b_hh fold (phn).  Software pipelined: chunk i+1's x-side GEMMs are
emitted before chunk i's h-side so PE never idles.  All matmuls bf16.
Logits are classified incrementally as columns retire.
"""

import contextlib

import numpy as np

import concourse.bacc as bacc
import concourse.mybir as mybir
import concourse.tile as tile
from concourse.bass_utils import run_bass_kernel_spmd

F32 = mybir.dt.float32
F32R = mybir.dt.float32r
BF16 = mybir.dt.bfloat16
AF = mybir.ActivationFunctionType
OP = mybir.AluOpType
NPBF16 = mybir.dt.np(BF16)

NCORES = 8
B, L, H, C = 4096, 8, 256, 8
NB = B // NCORES            # paths per core
G = 3 * H                   # gate width 768
DP, DA, DV = 128, 64, 32
KAV = 101                   # author 64 + venue 32 + onehot 3 + inv 1 + ones 1
R_OH, R_INV, R_ONE = 96, 99, 100
BIGZ = 30000.0              # sigmoid(-(x+BIGZ)) == 0.0 exactly in fp32
CH = 256                    # column chunk width


def default_caps():
    return [512, 448, 384, 320, 256, 192, 128, 64]


def chunks_of(caps):
    """(l, start-in-caps-layout, width) per stream + its gt2 block start.
    gt2 interleaves per-chunk [paper w | av w] blocks so one contiguous
    DMA carries a chunk's whole x-side (single completion semaphore)."""
    bases = np.concatenate([[0], np.cumsum(caps)]).astype(int)
    out = []
    for l, cap in enumerate(caps):
        c0 = 0
        for w in stream_widths(l, cap):
            g0 = int(bases[l]) + c0
            out.append((l, g0, w))
            c0 += w
    return out


def stream_widths(l, cap):
    """Column-stream widths for one step: <=256 wide early (throughput),
    <=128 for the late latency-bound steps (independent streams pipeline
    across steps since the recurrence is per-column)."""
    wmax = CH if l < 4 else 128
    out = []
    rem = cap
    while rem > 0:
        w = min(wmax, rem)
        out.append(w)
        rem -= w
    return out


def build_nc(caps=None, loop=0):
    """loop>0 wraps the whole body in a device-side For_i executing it
    `loop` times back-to-back - used only for wall-clock timing."""
    caps = list(caps) if caps is not None else default_caps()
    assert caps[0] == NB and all(c % 64 == 0 for c in caps)
    bases = np.concatenate([[0], np.cumsum(caps)]).astype(int)
    SP = int(bases[-1])
    chunks = chunks_of(caps)

    nc = bacc.Bacc("TRN2", target_bir_lowering=False, debug=False,
                   num_devices=NCORES)

    def din(name, shape, dt):
        return nc.dram_tensor(name, shape, dt, kind="ExternalInput").ap()

    gt2_d = din("gt2", [DP, 2 * SP], BF16)   # per-chunk [paper w | av w]
    wx_d = din("wx", [DP, 2 * G], BF16)      # wc1 | lhs2 (zero-padded rows)
    whh_d = din("whh", [128, 2 * G], BF16)   # W_hh.T halves side by side
    swc_d = din("swc", [128, 2 * C], BF16)   # Wc.T halves side by side
    bhb_d = din("bhb", [128, 7], F32)        # bhh6 | bc (rows 0:C of col 6)
    bhr_d = din("bhr", [33, 128], BF16)      # b_hh n-gate rows at 0 and 32
    out_d = nc.dram_tensor("logitsT", [C, NB], F32, kind="ExternalOutput").ap()

    with tile.TileContext(nc) as tc:
        pers = tc.alloc_tile_pool(name="pers", bufs=1)

        def T(shape, dt, name):
            return pers.tile(shape, dt, tag=name, name=name)

        gt2 = T([DP, 2 * SP], BF16, "gt2")
        wx = T([DP, 2 * G], BF16, "wx")       # wc1 | lhs2
        swhh2 = T([128, 2 * G], BF16, "swhh2")
        swc2 = T([128, 2 * C], BF16, "swc2")
        bhb = T([128, 7], F32, "bhb")
        bhr = T([33, 128], BF16, "bhr")
        ones = T([33, NB], BF16, "ones")
        zlh = T([1, 512], BF16, "zlh")
        hT = T([128, 2 * NB], BF16, "hT")     # cols 0:512 mt0 | 512:1024 mt1
        lsb = T([C, NB], F32, "lsb")

        with (
            tc.tile_pool(name="pr", bufs=2, space="PSUM") as ppr,
            tc.tile_pool(name="pz", bufs=2, space="PSUM") as ppz,
            tc.tile_pool(name="phn", bufs=2, space="PSUM") as pphn,
            tc.tile_pool(name="pxn", bufs=2, space="PSUM") as ppxn,
            tc.tile_pool(name="gate", bufs=4) as gp,
        ):
            # ------ engine warmup: PE p-state ramp + ACT table load + the
            # ------ constants the recurrence needs (before any timing loop)
            with tc.tile_pool(name="warm", bufs=1) as wp:
                wsrc = wp.tile([128, CH], F32R, tag="wsrc")
                wps = ppr.tile([128, 512], F32, tag="pr", name="warmp")
                wact = wp.tile([128, 16], F32, tag="wact")
                nc.vector.memset(wsrc[:].bitcast(F32), 0.0)
                nc.vector.memset(zlh[:], 0.0)
                nc.vector.memset(wact[:], 0.0)
                nc.vector.memset(ones[:], 1.0)
                nc.scalar.activation(wact[:], wact[:], AF.Sigmoid)
                nc.scalar.activation(wact[:], wact[:], AF.Tanh)
                nc.gpsimd.tensor_add(wact[:], wact[:], wact[:])
                for w in range(14):
                    nc.tensor.matmul(wps[:, 0:CH], wsrc[:, 0:128], wsrc[:],
                                     start=(w == 0), stop=(w == 13))

            if True:
                # -------- input DMAs: gt2 + wx on the SP/HWDGE queue in
                # priority order; later weights descriptor-gen on Pool
                def gt_dma(c0, c1):
                    sl = slice(2 * int(c0), 2 * int(c1))
                    nc.sync.dma_start(gt2[:, sl], gt2_d[:, sl])

                nc.gpsimd.dma_start(swc2[:], swc_d[:, :])
                nc.gpsimd.dma_start(bhb[:], bhb_d[:, :])
                nc.gpsimd.dma_start(bhr[:], bhr_d[:, :])
                nc.sync.dma_start(wx[:, 0:G], wx_d[:, 0:G])
                nc.sync.dma_start(wx[:, G:2 * G], wx_d[:, G:2 * G])
                gt_dma(0, CH)
                gt_dma(CH, bases[1])
                nc.sync.dma_start(swhh2[:], whh_d[:, :])
                gt_dma(bases[1], bases[min(3, len(caps))])
                if len(caps) > 3:
                    gt_dma(bases[3], SP)

                # -------- recurrence, software pipelined.  One PSUM
                # accumulation group per bank (zero regions are 2KB/bank),
                # opened by each tile's first matmul, whose operands arrive
                # via a single DMA semaphore so emission order holds.
                def p_open(p, w):
                    nc.tensor.matmul(p[0:128, 0:w], zlh[0:1, 0:128],
                                     zlh[0:1, 0:w], start=True, stop=False)

                def emit_x(i):
                    l, g0, w = chunks[i]
                    cs = slice(2 * g0, 2 * g0 + w)
                    av = slice(2 * g0 + w, 2 * g0 + 2 * w)
                    pr = ppr.tile([128, 512], F32, tag="pr", name=f"pr{i}")
                    pz = ppz.tile([128, 512], F32, tag="pz", name=f"pz{i}")
                    pxn = ppxn.tile([128, 512], F32, tag="pxn", name=f"px{i}")
                    for p in (pr, pz, pxn):
                        p_open(p, w)
                    for half in range(2):
                        po = slice(half * w, (half + 1) * w)
                        for gi, p in ((0, pr), (1, pz), (2, pxn)):
                            m0 = (2 * gi + half) * 128
                            last = half == 1 and (l == 0 or gi == 2)
                            nc.tensor.matmul(p[:, po], wx[:, m0:m0 + 128],
                                             gt2[:, cs], start=False,
                                             stop=False)
                            nc.tensor.matmul(p[:, po],
                                             wx[0:KAV, G + m0:G + m0 + 128],
                                             gt2[0:KAV, av], start=False,
                                             stop=last)
                    return pr, pz, pxn

                def emit_h_gates(i, pr, pz, pxn):
                    l, g0, w = chunks[i]
                    c0 = g0 - int(bases[l])          # path-column offset
                    hs = [slice(k * NB + c0, k * NB + c0 + w)
                          for k in range(2)]
                    phn = None
                    if l > 0:
                        phn = pphn.tile([128, 512], F32, tag="phn",
                                        name=f"ph{i}")
                        p_open(phn, w)
                        # r first (heads the gate chain), then n (feeds
                        # tt), z last (consumed latest)
                        for gi, p in ((0, pr), (2, phn), (1, pz)):
                            for half in range(2):
                                po = slice(half * w, (half + 1) * w)
                                m0 = (2 * gi + half) * 128
                                for k in range(2):
                                    nc.tensor.matmul(
                                        p[:, po],
                                        swhh2[:,
                                              k * G + m0:k * G + m0 + 128],
                                        hT[:, hs[k]],
                                        start=False,
                                        stop=(k == 1 and half == 1
                                              and gi != 2))
                                if gi == 2:
                                    nc.tensor.matmul(
                                        phn[:, po],
                                        bhr[32 * half:32 * half + 1, :],
                                        ones[32 * half:32 * half + 1, 0:w],
                                        start=False, stop=(half == 1))
                    rr = gp.tile([128, 512], BF16, tag="rr", name=f"rr{i}")
                    zz = gp.tile([128, 512], BF16, tag="zz", name=f"zz{i}")
                    tt = gp.tile([128, 512], BF16, tag="tt", name=f"tt{i}")
                    nn = gp.tile([128, 512], BF16, tag="nn", name=f"nn{i}")
                    po = [slice(h_ * w, (h_ + 1) * w) for h_ in range(2)]
                    p2 = slice(0, 2 * w)
                    nc.scalar.activation(rr[:, p2], pr[:, p2], AF.Sigmoid)
                    nc.scalar.activation(zz[:, p2], pz[:, p2], AF.Sigmoid,
                                         scale=-1.0)
                    if l > 0:
                        nc.vector.tensor_mul(tt[:, p2], phn[:, p2],
                                             rr[:, p2])
                    else:
                        for h_ in range(2):
                            nc.vector.tensor_scalar_mul(
                                tt[:, po[h_]], rr[:, po[h_]],
                                bhb[:, 4 + h_:5 + h_])
                    nc.vector.tensor_add(pxn[:, p2], pxn[:, p2], tt[:, p2])
                    nc.scalar.activation(nn[:, p2], pxn[:, p2], AF.Tanh)
                    # h' = h + z'*(n - h)   (z' = 1-z); at l=0: h = z'*n
                    tm = gp.tile([128, 512], BF16, tag="tm", name=f"tm{i}")
                    for h_, eng in ((0, nc.vector), (1, nc.vector)):
                        p_ = po[h_]
                        hv = hT[:, h_ * NB + c0:h_ * NB + c0 + w]
                        if l == 0:
                            eng.tensor_mul(hv, zz[:, p_], nn[:, p_])
                        else:
                            eng.tensor_sub(tm[:, p_], nn[:, p_], hv)
                            eng.tensor_mul(tm[:, p_], zz[:, p_], tm[:, p_])
                            eng.tensor_add(hv, hv, tm[:, p_])

                # classify columns [lo, hi) whose h became final at step l
                def emit_cls(lo, hi):
                    pl = ppr.tile([128, 512], F32, tag="pr", name=f"pl{lo}")
                    for k in range(2):
                        nc.tensor.matmul(pl[0:C, lo:hi],
                                         swc2[:, k * C:(k + 1) * C],
                                         hT[:, k * NB + lo:k * NB + hi],
                                         start=(k == 0), stop=(k == 1))
                    nc.vector.tensor_scalar(lsb[:, lo:hi], pl[0:C, lo:hi],
                                            bhb[0:C, 6:7], None, op0=OP.add)
                    nc.sync.dma_start(out_d[:, lo:hi], lsb[:, lo:hi])

                caps1 = caps + [0]
                pend = emit_x(0)
                for i in range(len(chunks)):
                    nxt = emit_x(i + 1) if i + 1 < len(chunks) else None
                    emit_h_gates(i, *pend)
                    pend = nxt
                    l = chunks[i][0]
                    last = i + 1 == len(chunks) or chunks[i + 1][0] != l
                    if not last:
                        continue
                    if len(caps) < 5:    # degenerate: one classify at end
                        if i + 1 == len(chunks):
                            emit_cls(0, NB)
                    elif l == 3 and caps1[4] < NB:
                        emit_cls(caps1[4], NB)
                    elif l > 3 and caps1[l + 1] < caps1[l]:
                        emit_cls(caps1[l + 1], caps1[l])
                    elif i + 1 == len(chunks):
                        emit_cls(0, caps1[l])

        pers.release()

    nc.finalize()
    return nc


# ---------------------------------------------------------------- host side

def compute_caps(lengths, perm):
    maxc = np.zeros(L, np.int64)
    for c in range(NCORES):
        plen = lengths[perm[c::NCORES]]
        for l in range(L):
            maxc[l] = max(maxc[l], int((plen > l).sum()))
    caps = [NB]
    for l in range(1, L):
        caps.append(int(min(NB, 64 * -(-maxc[l] // 64))))
    while caps and caps[-1] == 0:
        caps.pop()
    return caps


def make_in_maps(inputs):
    f32 = lambda k: np.asarray(inputs[k], dtype=np.float32)
    i64 = lambda k: np.asarray(inputs[k]).astype(np.int64)
    W_ih, W_hh = f32("W_ih"), f32("W_hh")
    b_ih, b_hh = f32("b_ih"), f32("b_hh")
    paper, author, venue = f32("paper_x"), f32("author_x"), f32("venue_x")
    lengths, type_ids, node_ids = (i64("lengths"), i64("type_ids"),
                                   i64("node_ids"))

    perm = np.argsort(-lengths, kind="stable")
    caps = compute_caps(lengths, perm)
    bases = np.concatenate([[0], np.cumsum(caps)]).astype(int)
    SP = int(bases[-1])

    wx = np.zeros((DP, 2 * G), np.float32)
    wx[:, 0:G] = (W_ih @ f32("Wp")).T
    wx[0:DA, G:2 * G] = (W_ih @ f32("Wa")).T
    wx[DA:DA + DV, G:2 * G] = (W_ih @ f32("Wv")).T
    for t, bk in enumerate(("bp", "ba", "bv")):
        wx[R_OH + t, G:2 * G] = W_ih @ f32(bk) + b_ih
    wx[R_INV, G + H:G + 2 * H] = BIGZ
    wx[R_ONE, G:G + 2 * H] = b_hh[0:2 * H]
    whh = np.zeros((128, 2 * G), np.float32)
    whh[:, 0:G] = W_hh.T[0:128]
    whh[:, G:2 * G] = W_hh.T[128:256]
    swc = np.zeros((128, 2 * C), np.float32)
    swc[:, 0:C] = f32("Wc").T[0:128]
    swc[:, C:2 * C] = f32("Wc").T[128:256]
    bhr = np.zeros((33, 128), np.float32)
    bhr[0] = b_hh[2 * H:2 * H + 128]
    bhr[32] = b_hh[2 * H + 128:3 * H]
    bhb = np.zeros((128, 7), np.float32)
    bhb[:, 0:6] = b_hh.reshape(6, 128).T
    bhb[0:C, 6] = f32("bc")
    shared = {
        "wx": np.ascontiguousarray(wx.astype(NPBF16)),
        "whh": np.ascontiguousarray(whh.astype(NPBF16)),
        "swc": np.ascontiguousarray(swc.astype(NPBF16)),
        "bhb": np.ascontiguousarray(bhb),
        "bhr": np.ascontiguousarray(bhr.astype(NPBF16)),
    }

    in_maps = []
    for c in range(NCORES):
        pids = perm[c::NCORES]               # original path ids, desc length
        plen = lengths[pids]
        gtp = np.zeros((SP, DP), np.float32)     # slot-major; transposed below
        gtav = np.zeros((SP, KAV), np.float32)
        for l in range(len(caps)):
            b0, cap = int(bases[l]), caps[l]
            idx = pids[:cap]
            valid = plen[:cap] > l
            t = type_ids[idx, l]
            nid = node_ids[idx, l]
            rows = b0 + np.arange(cap)
            pm = valid & (t == 0)
            am = valid & (t == 1)
            vm = valid & (t == 2)
            gtp[rows[pm]] = paper[nid[pm]]
            gtav[rows[am], 0:DA] = author[nid[am]]
            gtav[rows[vm], DA:DA + DV] = venue[nid[vm]]
            gtav[rows, R_OH] = pm
            gtav[rows, R_OH + 1] = am
            gtav[rows, R_OH + 2] = vm
            gtav[rows, R_INV] = ~valid
            gtav[rows, R_ONE] = 1.0
        m = dict(shared)
        g2 = np.zeros((DP, 2 * SP), np.float32)
        for (_, g0, w) in chunks_of(caps):
            g2[:, 2 * g0:2 * g0 + w] = gtp.T[:, g0:g0 + w]
            g2[0:KAV, 2 * g0 + w:2 * g0 + 2 * w] = gtav.T[:, g0:g0 + w]
        m["gt2"] = np.ascontiguousarray(g2.astype(NPBF16))
        in_maps.append(m)
    return in_maps, perm, caps


_NC_CACHE = {}


def _get_nc(caps=None, loop=0):
    key = (tuple(caps) if caps is not None else tuple(default_caps()), loop)
    if key not in _NC_CACHE:
        _NC_CACHE[key] = build_nc(list(key[0]), loop=loop)
    return _NC_CACHE[key]


def kernel(**inputs) -> np.ndarray:
    in_maps, perm, caps = make_in_maps(inputs)
    nc = _get_nc(caps)
    res = run_bass_kernel_spmd(nc, in_maps, core_ids=list(range(NCORES)))
    out = np.empty((B, C), np.float32)
    for c in range(NCORES):
        lt = np.asarray(res.results[c]["logitsT"])    # [C, NB]
        out[perm[c::NCORES]] = lt.T
    return np.ascontiguousarray(out)


# revision 77
# speedup vs baseline: 1.0355x; 1.0240x over previous
"""Trainium2 Bass kernel for MetaPathClassifier (heterogeneous-path GRU).

Strategy (data-parallel over 8 cores, 512 paths each):

Host prep: sort paths by length (desc) and deal round-robin so every core
sees a near-identical length profile; compute per-step column capacities
caps[l] (multiples of 64, shared across cores).  Gather + type-split +
transpose the node features on the host into one feature-major table
  gt2 [128, 2*S']: cols 0:S' paper features (zero for non-paper slots);
  cols S':2S' rows 0:101 = author 0:64 | venue 64:96 | one-hot type
  96:99 | invalid flag 99 | ones 100.
Step-major layout: step l owns columns [base_l, base_l+caps[l]).
Combined weights (W_ih @ W_t).T, per-type bias rows (W_ih@b_t + b_ih), a
BIG-z row (invalid slots saturate the z gate so h freezes exactly), and a
b_hh row (via the ones-row) are extra contraction rows, so the r/z gate
pre-activations come out of PSUM complete - no bias adds on the gate path.

Device: per column stream of each step (<=256 wide early, <=128 late so
independent streams pipeline across steps), x-side and h-side GEMMs
accumulate into one PSUM bank per gate pair (m-tile halves side by side),
ACT applies sigmoid/tanh straight from PSUM, DVE forms the n-gate product
and the convex h update h += z'*(n-h) in bf16 (2x mode).  One PSUM
accumulation group per bank (zero regions are bank-granular), opened by
the first x matmul (all x operands arrive via one DMA semaphore) or the
rank-1 b_hh fold (phn).  Software pipelined: chunk i+1's x-side GEMMs are
emitted before chunk i's h-side so PE never idles.  All matmuls bf16.
Logits are classified incrementally as columns retire.
"""

import contextlib

import numpy as np

import concourse.bacc as bacc
import concourse.mybir as mybir
import concourse.tile as tile
from concourse.bass_utils import run_bass_kernel_spmd

F32 = mybir.dt.float32
F32R = mybir.dt.float32r
BF16 = mybir.dt.bfloat16
AF = mybir.ActivationFunctionType
OP = mybir.AluOpType
NPBF16 = mybir.dt.np(BF16)

NCORES = 8
B, L, H, C = 4096, 8, 256, 8
NB = B // NCORES            # paths per core
G = 3 * H                   # gate width 768
DP, DA, DV = 128, 64, 32
KAV = 101                   # author 64 + venue 32 + onehot 3 + inv 1 + ones 1
R_OH, R_INV, R_ONE = 96, 99, 100
BIGZ = 30000.0              # sigmoid(-(x+BIGZ)) == 0.0 exactly in fp32
CH = 256                    # column chunk width


def default_caps():
    return [512, 448, 384, 320, 256, 192, 128, 64]


def chunks_of(caps):
    """(l, start-in-caps-layout, width) per stream + its gt2 block start.
    gt2 interleaves per-chunk [paper w | av w] blocks so one contiguous
    DMA carries a chunk's whole x-side (single completion semaphore)."""
    bases = np.concatenate([[0], np.cumsum(caps)]).astype(int)
    out = []
    for l, cap in enumerate(caps):
        c0 = 0
        for w in stream_widths(l, cap):
            g0 = int(bases[l]) + c0
            out.append((l, g0, w))
            c0 += w
    return out


def stream_widths(l, cap):
    """Column-stream widths for one step: <=256 wide early (throughput),
    <=128 for the late latency-bound steps (independent streams pipeline
    across steps since the recurrence is per-column)."""
    wmax = CH if l < 4 else 128
    out = []
    rem = cap
    while rem > 0:
        w = min(wmax, rem)
        out.append(w)
        rem -= w
    return out


def build_nc(caps=None, loop=0):
    """loop>0 wraps the whole body in a device-side For_i executing it
    `loop` times back-to-back - used only for wall-clock timing."""
    caps = list(caps) if caps is not None else default_caps()
    assert caps[0] == NB and all(c % 64 == 0 for c in caps)
    bases = np.concatenate([[0], np.cumsum(caps)]).astype(int)
    SP = int(bases[-1])
    chunks = chunks_of(caps)

    nc = bacc.Bacc("TRN2", target_bir_lowering=False, debug=False,
                   num_devices=NCORES)

    def din(name, shape, dt):
        return nc.dram_tensor(name, shape, dt, kind="ExternalInput").ap()

    gt2_d = din("gt2", [DP, 2 * SP], BF16)   # per-chunk [paper w | av w]
    wx_d = din("wx", [DP, 2 * G], BF16)      # wc1 | lhs2 (zero-padded rows)
    whh_d = din("whh", [128, 2 * G], BF16)   # W_hh.T halves side by side
    swc_d = din("swc", [128, 2 * C], BF16)   # Wc.T halves side by side
    bhb_d = din("bhb", [128, 7], F32)        # bhh6 | bc (rows 0:C of col 6)
    bhr_d = din("bhr", [33, 128], BF16)      # b_hh n-gate rows at 0 and 32
    out_d = nc.dram_tensor("logitsT", [C, NB], F32, kind="ExternalOutput").ap()

    with tile.TileContext(nc) as tc:
        pers = tc.alloc_tile_pool(name="pers", bufs=1)

        def T(shape, dt, name):
            return pers.tile(shape, dt, tag=name, name=name)

        gt2 = T([DP, 2 * SP], BF16, "gt2")
        wx = T([DP, 2 * G], BF16, "wx")       # wc1 | lhs2
        swhh2 = T([128, 2 * G], BF16, "swhh2")
        swc2 = T([128, 2 * C], BF16, "swc2")
        bhb = T([128, 7], F32, "bhb")
        bhr = T([33, 128], BF16, "bhr")
        ones = T([33, NB], BF16, "ones")
        zlh = T([1, 512], BF16, "zlh")
        hT = T([128, 2 * NB], BF16, "hT")     # cols 0:512 mt0 | 512:1024 mt1
        lsb = T([C, NB], F32, "lsb")

        with (
            tc.tile_pool(name="pr", bufs=2, space="PSUM") as ppr,
            tc.tile_pool(name="pz", bufs=2, space="PSUM") as ppz,
            tc.tile_pool(name="phn", bufs=2, space="PSUM") as pphn,
            tc.tile_pool(name="pxn", bufs=2, space="PSUM") as ppxn,
            tc.tile_pool(name="gate", bufs=4) as gp,
        ):
            # ------ engine warmup: PE p-state ramp + ACT table load + the
            # ------ constants the recurrence needs (before any timing loop)
            with tc.tile_pool(name="warm", bufs=1) as wp:
                wsrc = wp.tile([128, CH], F32R, tag="wsrc")
                wps = ppr.tile([128, 512], F32, tag="pr", name="warmp")
                wact = wp.tile([128, 16], F32, tag="wact")
                nc.vector.memset(wsrc[:].bitcast(F32), 0.0)
                nc.vector.memset(zlh[:], 0.0)
                nc.vector.memset(wact[:], 0.0)
                nc.vector.memset(ones[:], 1.0)
                nc.scalar.activation(wact[:], wact[:], AF.Sigmoid)
                nc.scalar.activation(wact[:], wact[:], AF.Tanh)
                nc.gpsimd.tensor_add(wact[:], wact[:], wact[:])
                for w in range(14):
                    nc.tensor.matmul(wps[:, 0:CH], wsrc[:, 0:128], wsrc[:],
                                     start=(w == 0), stop=(w == 13))

            if True:
                # -------- input DMAs: gt2 + wx on the SP/HWDGE queue in
                # priority order; later weights descriptor-gen on Pool
                def gt_dma(c0, c1):
                    sl = slice(2 * int(c0), 2 * int(c1))
                    nc.sync.dma_start(gt2[:, sl], gt2_d[:, sl])

                nc.gpsimd.dma_start(swc2[:], swc_d[:, :])
                nc.gpsimd.dma_start(bhb[:], bhb_d[:, :])
                nc.gpsimd.dma_start(bhr[:], bhr_d[:, :])
                nc.sync.dma_start(wx[:, 0:G], wx_d[:, 0:G])
                nc.sync.dma_start(wx[:, G:2 * G], wx_d[:, G:2 * G])
                gt_dma(0, CH)
                gt_dma(CH, bases[1])
                nc.sync.dma_start(swhh2[:], whh_d[:, :])
                gt_dma(bases[1], bases[min(3, len(caps))])
                if len(caps) > 3:
                    gt_dma(bases[3], SP)

                # -------- recurrence, software pipelined.  One PSUM
                # accumulation group per bank (zero regions are 2KB/bank),
                # opened by each tile's first matmul, whose operands arrive
                # via a single DMA semaphore so emission order holds.
                def p_open(p, w):
                    nc.tensor.matmul(p[0:128, 0:w], zlh[0:1, 0:128],
                                     zlh[0:1, 0:w], start=True, stop=False)

                def emit_x(i):
                    l, g0, w = chunks[i]
                    cs = slice(2 * g0, 2 * g0 + w)
                    av = slice(2 * g0 + w, 2 * g0 + 2 * w)
                    pr = ppr.tile([128, 512], F32, tag="pr", name=f"pr{i}")
                    pz = ppz.tile([128, 512], F32, tag="pz", name=f"pz{i}")
                    pxn = ppxn.tile([128, 512], F32, tag="pxn", name=f"px{i}")
                    for half in range(2):
                        po = slice(half * w, (half + 1) * w)
                        for gi, p in ((0, pr), (1, pz), (2, pxn)):
                            m0 = (2 * gi + half) * 128
                            last = half == 1 and (l == 0 or gi == 2)
                            nc.tensor.matmul(p[:, po], wx[:, m0:m0 + 128],
                                             gt2[:, cs], start=(half == 0),
                                             stop=False)
                            nc.tensor.matmul(p[:, po],
                                             wx[0:KAV, G + m0:G + m0 + 128],
                                             gt2[0:KAV, av], start=False,
                                             stop=last)
                    return pr, pz, pxn

                def emit_h_gates(i, pr, pz, pxn):
                    l, g0, w = chunks[i]
                    c0 = g0 - int(bases[l])          # path-column offset
                    hs = [slice(k * NB + c0, k * NB + c0 + w)
                          for k in range(2)]
                    phn = None
                    if l > 0:
                        phn = pphn.tile([128, 512], F32, tag="phn",
                                        name=f"ph{i}")
                        p_open(phn, w)
                        # r first (heads the gate chain), then n (feeds
                        # tt), z last (consumed latest)
                        for gi, p in ((0, pr), (2, phn), (1, pz)):
                            for half in range(2):
                                po = slice(half * w, (half + 1) * w)
                                m0 = (2 * gi + half) * 128
                                for k in range(2):
                                    nc.tensor.matmul(
                                        p[:, po],
                                        swhh2[:,
                                              k * G + m0:k * G + m0 + 128],
                                        hT[:, hs[k]],
                                        start=False,
                                        stop=(k == 1 and half == 1
                                              and gi != 2))
                                if gi == 2:
                                    nc.tensor.matmul(
                                        phn[:, po],
                                        bhr[32 * half:32 * half + 1, :],
                                        ones[32 * half:32 * half + 1, 0:w],
                                        start=False, stop=(half == 1))
                    rr = gp.tile([128, 512], BF16, tag="rr", name=f"rr{i}")
                    zz = gp.tile([128, 512], BF16, tag="zz", name=f"zz{i}")
                    tt = gp.tile([128, 512], BF16, tag="tt", name=f"tt{i}")
                    nn = gp.tile([128, 512], BF16, tag="nn", name=f"nn{i}")
                    po = [slice(h_ * w, (h_ + 1) * w) for h_ in range(2)]
                    p2 = slice(0, 2 * w)
                    nc.scalar.activation(rr[:, p2], pr[:, p2], AF.Sigmoid)
                    nc.scalar.activation(zz[:, p2], pz[:, p2], AF.Sigmoid,
                                         scale=-1.0)
                    if l > 0:
                        nc.vector.tensor_mul(tt[:, p2], phn[:, p2],
                                             rr[:, p2])
                    else:
                        for h_ in range(2):
                            nc.vector.tensor_scalar_mul(
                                tt[:, po[h_]], rr[:, po[h_]],
                                bhb[:, 4 + h_:5 + h_])
                    nc.vector.tensor_add(pxn[:, p2], pxn[:, p2], tt[:, p2])
                    nc.scalar.activation(nn[:, p2], pxn[:, p2], AF.Tanh)
                    # h' = h + z'*(n - h)   (z' = 1-z); at l=0: h = z'*n
                    tm = gp.tile([128, 512], BF16, tag="tm", name=f"tm{i}")
                    for h_, eng in ((0, nc.vector), (1, nc.vector)):
                        p_ = po[h_]
                        hv = hT[:, h_ * NB + c0:h_ * NB + c0 + w]
                        if l == 0:
                            eng.tensor_mul(hv, zz[:, p_], nn[:, p_])
                        else:
                            eng.tensor_sub(tm[:, p_], nn[:, p_], hv)
                            eng.tensor_mul(tm[:, p_], zz[:, p_], tm[:, p_])
                            eng.tensor_add(hv, hv, tm[:, p_])

                # classify columns [lo, hi) whose h became final at step l
                def emit_cls(lo, hi):
                    pl = ppr.tile([128, 512], F32, tag="pr", name=f"pl{lo}")
                    for k in range(2):
                        nc.tensor.matmul(pl[0:C, lo:hi],
                                         swc2[:, k * C:(k + 1) * C],
                                         hT[:, k * NB + lo:k * NB + hi],
                                         start=(k == 0), stop=(k == 1))
                    nc.vector.tensor_scalar(lsb[:, lo:hi], pl[0:C, lo:hi],
                                            bhb[0:C, 6:7], None, op0=OP.add)
                    nc.sync.dma_start(out_d[:, lo:hi], lsb[:, lo:hi])

                caps1 = caps + [0]
                pend = emit_x(0)
                for i in range(len(chunks)):
                    nxt = emit_x(i + 1) if i + 1 < len(chunks) else None
                    emit_h_gates(i, *pend)
                    pend = nxt
                    l = chunks[i][0]
                    last = i + 1 == len(chunks) or chunks[i + 1][0] != l
                    if not last:
                        continue
                    if len(caps) < 5:    # degenerate: one classify at end
                        if i + 1 == len(chunks):
                            emit_cls(0, NB)
                    elif l == 3 and caps1[4] < NB:
                        emit_cls(caps1[4], NB)
                    elif l > 3 and caps1[l + 1] < caps1[l]:
                        emit_cls(caps1[l + 1], caps1[l])
                    elif i + 1 == len(chunks):
                        emit_cls(0, caps1[l])

        pers.release()

    nc.finalize()
    return nc


# ---------------------------------------------------------------- host side

def compute_caps(lengths, perm):
    maxc = np.zeros(L, np.int64)
    for c in range(NCORES):
        plen = lengths[perm[c::NCORES]]
        for l in range(L):
            maxc[l] = max(maxc[l], int((plen > l).sum()))
    caps = [NB]
    for l in range(1, L):
        caps.append(int(min(NB, 64 * -(-maxc[l] // 64))))
    while caps and caps[-1] == 0:
        caps.pop()
    return caps


def make_in_maps(inputs):
    f32 = lambda k: np.asarray(inputs[k], dtype=np.float32)
    i64 = lambda k: np.asarray(inputs[k]).astype(np.int64)
    W_ih, W_hh = f32("W_ih"), f32("W_hh")
    b_ih, b_hh = f32("b_ih"), f32("b_hh")
    paper, author, venue = f32("paper_x"), f32("author_x"), f32("venue_x")
    lengths, type_ids, node_ids = (i64("lengths"), i64("type_ids"),
                                   i64("node_ids"))

    perm = np.argsort(-lengths, kind="stable")
    caps = compute_caps(lengths, perm)
    bases = np.concatenate([[0], np.cumsum(caps)]).astype(int)
    SP = int(bases[-1])

    wx = np.zeros((DP, 2 * G), np.float32)
    wx[:, 0:G] = (W_ih @ f32("Wp")).T
    wx[0:DA, G:2 * G] = (W_ih @ f32("Wa")).T
    wx[DA:DA + DV, G:2 * G] = (W_ih @ f32("Wv")).T
    for t, bk in enumerate(("bp", "ba", "bv")):
        wx[R_OH + t, G:2 * G] = W_ih @ f32(bk) + b_ih
    wx[R_INV, G + H:G + 2 * H] = BIGZ
    wx[R_ONE, G:G + 2 * H] = b_hh[0:2 * H]
    whh = np.zeros((128, 2 * G), np.float32)
    whh[:, 0:G] = W_hh.T[0:128]
    whh[:, G:2 * G] = W_hh.T[128:256]
    swc = np.zeros((128, 2 * C), np.float32)
    swc[:, 0:C] = f32("Wc").T[0:128]
    swc[:, C:2 * C] = f32("Wc").T[128:256]
    bhr = np.zeros((33, 128), np.float32)
    bhr[0] = b_hh[2 * H:2 * H + 128]
    bhr[32] = b_hh[2 * H + 128:3 * H]
    bhb = np.zeros((128, 7), np.float32)
    bhb[:, 0:6] = b_hh.reshape(6, 128).T
    bhb[0:C, 6] = f32("bc")
    shared = {
        "wx": np.ascontiguousarray(wx.astype(NPBF16)),
        "whh": np.ascontiguousarray(whh.astype(NPBF16)),
        "swc": np.ascontiguousarray(swc.astype(NPBF16)),
        "bhb": np.ascontiguousarray(bhb),
        "bhr": np.ascontiguousarray(bhr.astype(NPBF16)),
    }

    in_maps = []
    for c in range(NCORES):
        pids = perm[c::NCORES]               # original path ids, desc length
        plen = lengths[pids]
        gtp = np.zeros((SP, DP), np.float32)     # slot-major; transposed below
        gtav = np.zeros((SP, KAV), np.float32)
        for l in range(len(caps)):
            b0, cap = int(bases[l]), caps[l]
            idx = pids[:cap]
            valid = plen[:cap] > l
            t = type_ids[idx, l]
            nid = node_ids[idx, l]
            rows = b0 + np.arange(cap)
            pm = valid & (t == 0)
            am = valid & (t == 1)
            vm = valid & (t == 2)
            gtp[rows[pm]] = paper[nid[pm]]
            gtav[rows[am], 0:DA] = author[nid[am]]
            gtav[rows[vm], DA:DA + DV] = venue[nid[vm]]
            gtav[rows, R_OH] = pm
            gtav[rows, R_OH + 1] = am
            gtav[rows, R_OH + 2] = vm
            gtav[rows, R_INV] = ~valid
            gtav[rows, R_ONE] = 1.0
        m = dict(shared)
        g2 = np.zeros((DP, 2 * SP), np.float32)
        for (_, g0, w) in chunks_of(caps):
            g2[:, 2 * g0:2 * g0 + w] = gtp.T[:, g0:g0 + w]
            g2[0:KAV, 2 * g0 + w:2 * g0 + 2 * w] = gtav.T[:, g0:g0 + w]
        m["gt2"] = np.ascontiguousarray(g2.astype(NPBF16))
        in_maps.append(m)
    return in_maps, perm, caps


_NC_CACHE = {}


def _get_nc(caps=None, loop=0):
    key = (tuple(caps) if caps is not None else tuple(default_caps()), loop)
    if key not in _NC_CACHE:
        _NC_CACHE[key] = build_nc(list(key[0]), loop=loop)
    return _NC_CACHE[key]


def kernel(**inputs) -> np.ndarray:
    in_maps, perm, caps = make_in_maps(inputs)
    nc = _get_nc(caps)
    res = run_bass_kernel_spmd(nc, in_maps, core_ids=list(range(NCORES)))
    out = np.empty((B, C), np.float32)
    for c in range(NCORES):
        lt = np.asarray(res.results[c]["logitsT"])    # [C, NB]
        out[perm[c::NCORES]] = lt.T
    return np.ascontiguousarray(out)
